# revision 35
# baseline (speedup 1.0000x reference)
"""Trainium2 Bass kernel for nn_BandSplit (v3).

Computes, for each of K mel bands:
    out[b, o, t, k] = sum_{c,w} x[b, c, t, idx[k,w]] * mel_w[k,w] * pre_w[k,c,w,o] + pre_b[k,o]

Structure exploited:
  - Band indices idx[k, :n_k] are contiguous runs (triangular mel filters),
    so the gather is a strided slice.
  - mel_w folds into pre_w on the host: W2[k,c,w,o] = mel_w[k,w]*pre_w[k,c,w,o].
  - x rows are channel-interleaved (row = 2f + c); band k's contraction is
    the contiguous row run [2s_k, 2s_k+2n_k).

v3 changes vs v2 (the 69-78us baseline):
  - The binding constraint in v2 was total DMA bytes through the slowest
    of the 16 DMA engines (engine 15 ran ~97% busy end to end).  Output
    moves to bf16 (rel-err gate is 2e-2; bf16 rounding adds ~1e-3): halves
    the dominant output stream (16.8 -> 8.4 MB/core).  Host upcasts.
  - x chunks are packed with bounded row duplication: when a band would
    cross a 128-row chunk boundary and the duplicated prefix is small, the
    chunk restarts at the band's first row instead of splitting the band
    into two matmuls.  94 -> ~86 matmuls, and weight zero-extension rows
    (split pieces force tile_position base 0) shrink.
  - The entire per-core output (64 KB/partition in bf16) is staged in SBUF
    (stage pool bufs = n_groups), so PE/DVE/ACT never block on the output
    stream; out-DMAs drain at the DMA engines' own pace.
  - Out-DMAs are one per 8 bands (8 KB/partition descriptors).
  - Warm-up memset runs on GpSimd (Pool) instead of DVE so the first
    PE warm-up matmul issues earlier; the HAM clock gate needs ~5us of
    sustained matmul activity before it lifts the PE 1.2 -> 2.4 GHz.

Sharding: data-parallel over T across 8 cores (T=1024 -> 128/core); identical
SPMD program per core, weights replicated, host reassembles (B, O, T, K).
"""

import os
import sys
import types

import numpy as np

for _p in ("/opt/trn_rl_repo",):
    if _p not in sys.path:
        sys.path.insert(0, _p)

import ml_dtypes

import concourse.bass as bass
import concourse.mybir as mybir
import concourse.tile as tile
from concourse import bass_utils

N_CORES = 8
O = 128          # out channels (= stationary free dim = PSUM partitions)
P = 128          # SBUF partitions / chunk rows
BT = 512         # B * T_loc columns per core
N_WARMUP = int(os.environ.get("BANDSPLIT_WARMUP", "3"))
DUP_MAX = int(os.environ.get("BANDSPLIT_DUPMAX", "32"))
# bands per stage tile / output DMA: small first groups align with the
# first input segment (positions 0-3) so the first group never waits on a
# later segment's transfer.
OUT_GROUPS = [4, 4] + [8] * 7
OPAD = int(os.environ.get("BANDSPLIT_OPAD", "64"))  # output row-stride pad (elems)
# Bands run big-first (descending): wide bands' full-array matmuls hold the
# HAM clock gate's utilization threshold from the start, so the 2.4 GHz
# flip happens early and most of the stream runs at full clock.
BAND_ORDER = lambda K: list(range(K - 1, -1, -1))
# Dummy full-array matmuls interleave after pairs of bands processed at
# positions < FILLER_POS: when the drain (PSUM->SBUF copies) paces the
# pipeline, the PE would idle ~35% at 2.4 GHz and the HAM gate would
# re-throttle it (measured: the down-flip lands exactly where the fillers
# stop); the filler keeps utilization pinned through the whole stream.
FILLER_POS = int(os.environ.get("BANDSPLIT_FILLER_POS", "32"))

_F32 = mybir.dt.float32
_IN_DT = mybir.dt.bfloat16
_IN_NP = ml_dtypes.bfloat16

if os.environ.get("BANDSPLIT_OUT_DT", "bf16") == "f32":
    _OUT_DT = mybir.dt.float32
    _OUT_NP = np.float32
else:
    _OUT_DT = mybir.dt.bfloat16
    _OUT_NP = ml_dtypes.bfloat16


# ---------------------------------------------------------------------------
# Workaround: this container's walrus rejects instructions carrying more than
# a couple of sem waits ("Too many sync wait commands", CoreV3GenImpl
# setupSyncWait).  Post-pass: move excess waits onto single-wait NoOps
# inserted just before the instruction on the same engine/sequencer.
# ---------------------------------------------------------------------------
_MAX_WAITS = 1

if os.environ.get("BANDSPLIT_LDWOPT"):
    # Experiment: let walrus overlap LDWEIGHTS with matmuls (the default
    # pipeline pins --enable-ldw-opt=false).
    _orig_run_command = bass_utils.run_command

    def _patched_run_command(cmd, **kw):
        if isinstance(cmd, list):
            cmd = [
                "--enable-ldw-opt=true" if c == "--enable-ldw-opt=false" else c
                for c in cmd
            ]
        return _orig_run_command(cmd, **kw)

    bass_utils.run_command = _patched_run_command


def _split_excess_waits(nc, max_waits=_MAX_WAITS):
    ctr = 0
    for f in nc.m.functions:
        for bb in f.blocks:
            il = bb.instructions
            i = 0
            while i < len(il):
                inst = il[i]
                si = inst.sync_info
                if si is not None and si.on_wait and len(si.on_wait) > max_waits:
                    waits = list(si.on_wait)
                    keep = waits[-max_waits:]
                    extra = waits[:-max_waits]
                    nops = []
                    for w in extra:
                        ctr += 1
                        nop = mybir.InstNoOp(
                            name=f"{inst.name}-wsplit{ctr}",
                            engine=inst.engine,
                            sync_info=mybir.SyncInfo(on_wait=[w], on_update=[]),
                            bass_nofuse=True,
                        )
                        nc.register_instruction(nop, overwrite=True)
                        nops.append(nop)
                    inst.sync_info = mybir.SyncInfo(
                        on_wait=keep, on_update=list(si.on_update or [])
                    )
                    il[i:i] = nops
                    i += len(nops)
                i += 1
    return ctr


# ---------------------------------------------------------------------------
# Optional NTFF profiling (test.py sets BANDSPLIT_TRACE=1).  The agent image's
# antenv lacks axon_hooks, so tracing degrades silently unless we install the
# ctypes-based hook ourselves.
# ---------------------------------------------------------------------------
def _install_trace_hook():
    try:
        import antenv  # noqa: F401
        from trn_agent_boot.trn_boot import _ntff_profile_via_ctypes

        if "antenv.axon_hooks" in sys.modules:
            return True
        hook = _ntff_profile_via_ctypes("/opt/axon/libaxon_pjrt.so")
        mod = types.ModuleType("antenv.axon_hooks")
        mod._hook = hook
        mod.get_axon_ntff_profile_hook = lambda: mod._hook
        mod.set_axon_ntff_profile_hook = lambda h: setattr(mod, "_hook", h)
        sys.modules["antenv.axon_hooks"] = mod
        import antenv as _ae

        _ae.axon_hooks = mod
        return True
    except Exception:
        return False


# ---------------------------------------------------------------------------
# Band structure extraction (host side, from the actual inputs)
# ---------------------------------------------------------------------------
def _band_structure(idx, mel_w):
    idx = np.asarray(idx)
    mel_w = np.asarray(mel_w)
    K = idx.shape[0]
    starts = np.empty(K, dtype=np.int64)
    lengths = np.empty(K, dtype=np.int64)
    for k in range(K):
        nz = np.nonzero(mel_w[k])[0]
        assert nz.size > 0, f"band {k} empty"
        n = int(nz.max()) + 1
        run = idx[k, :n]
        assert np.all(np.diff(run) == 1), f"band {k} indices not contiguous"
        starts[k] = int(run[0])
        lengths[k] = n
    return starts, lengths


def _align_base(p0, e):
    """Largest legal 32-aligned base <= p0 for a piece ending at e.

    tile_position rule: rows<=32 -> base in {0,32,64,96}; rows<=64 -> {0,64};
    rows>64 -> base 0.
    """
    for a in (96, 64, 32, 0):
        if a > p0:
            continue
        rows = e - a
        if rows <= 32 or (rows <= 64 and a in (0, 64)) or a == 0:
            return a
    raise AssertionError((p0, e))


# HW note: nonzero tile_position row bases are only safe for single-matmul
# bands (start=stop=True).  Mixing bases inside a PSUM accumulation group
# (split bands) aborts the NEFF at runtime on this stack, so split bands'
# pieces all use base 0 (with zero weight rows below p0).


def _plan(starts, lengths, F):
    """Pack band row-runs into 128-row chunks with bounded duplication.

    Chunks are arbitrary 128-row windows of the channel-interleaved row
    space, created in increasing start order.  A band crossing the current
    window restarts a fresh window at its own first row when the duplicated
    prefix is <= DUP_MAX rows (one matmul instead of two); otherwise it
    splits along the natural continuation grid.

    Returns:
      chunk_rows  -> list of chunk start rows (virtual row space, 2F wide)
      pieces[k]   -> list of (chunk, base, p0, e, wcol); rows [base,p0) are
                     zero weight extension, [p0,e) real
      n_wcol      -> number of packed 128-row weight columns
      wcol_first  -> first band using each weight column (ascending)
    """
    K = len(starts)
    chunks = []
    raw = []
    for k in range(K):
        a = 2 * int(starts[k])
        b = a + 2 * int(lengths[k])
        pl = []
        ci = None
        for i in range(len(chunks) - 1, -1, -1):
            if chunks[i] <= a:
                ci = i
                break
        if ci is None or a >= chunks[ci] + P:
            chunks.append(a)
            ci = len(chunks) - 1
        if b <= chunks[ci] + P:
            pl.append((ci, a - chunks[ci], b - a))
        else:
            dup = chunks[ci] + P - a
            if b - a <= P and dup <= DUP_MAX:
                chunks.append(a)
                ci = len(chunks) - 1
                pl.append((ci, 0, b - a))
            else:
                r = a
                while r < b:
                    if r >= chunks[ci] + P:
                        if ci + 1 < len(chunks) and chunks[ci + 1] <= r:
                            ci += 1
                        else:
                            chunks.append(chunks[ci] + P)
                            ci = len(chunks) - 1
                    e = min(b, chunks[ci] + P)
                    pl.append((ci, r - chunks[ci], e - r))
                    r = e
        raw.append(pl)

    # Weight column packing: first-fit on 32-row granules, in PROCESSING
    # order (descending bands), so early columns hold early-consumed
    # weights and column ranges stream in consumption order.
    #
    # Matmul rows are PADDED up to the 32-granule class (tgt): the HAM
    # clock gate tracks PE array utilization (rows streamed / 128), and
    # only sustained high-utilization matmul activity lifts the PE from
    # 1.2 to 2.4 GHz.  The pad rows are zeros already inside the piece's
    # reserved granules, so this costs no extra weight bytes.
    pieces = [[] for _ in range(K)]
    col_fill = []
    wcol_first = []
    for pos, k in enumerate(BAND_ORDER(K)):
        single = len(raw[k]) == 1
        for (c, p0, rows) in raw[k]:
            e = p0 + rows
            # Every matmul streams >=96 rows from base 0: measured on HW,
            # 96/128-row matmuls run 261ns at 2.4 GHz while 32/64-row ones
            # run 381ns AND fail to hold the HAM utilization gate.  The pad
            # rows are zero weight rows; base 0 keeps tile_position legal
            # for any row count.
            a0 = 0
            tgt = 96 if e <= 96 else P
            s_lo, s_hi = 0, tgt // 32
            wcol = None
            for j in range(len(col_fill)):
                if col_fill[j] <= s_lo:
                    wcol = j
                    col_fill[j] = s_hi
                    break
            if wcol is None:
                col_fill.append(s_hi)
                wcol_first.append(pos)
                wcol = len(col_fill) - 1
            pieces[k].append((c, a0, p0, e, wcol, tgt))
    return chunks, pieces, len(col_fill), wcol_first


# ---------------------------------------------------------------------------
# Device program
# ---------------------------------------------------------------------------
def _build_program(chunk_rows, pieces, n_wcol, wcol_first, K, with_bias):
    nc = bass.Bass("TRN2", target_bir_lowering=False, debug=False)
    n_xch = len(chunk_rows)
    # +OPAD on the row stride for the same HBM channel de-aliasing reason
    # as `out` below (n_xch*BT*2 bytes is an exact multiple of 4KB).
    xg = nc.dram_tensor(
        "xg", [P, n_xch * BT + OPAD], _IN_DT, kind="ExternalInput"
    ).ap()
    wg = nc.dram_tensor("wg", [P, n_wcol * O], _IN_DT, kind="ExternalInput").ap()
    if with_bias:
        bt = nc.dram_tensor("bt", [O, K], _F32, kind="ExternalInput").ap()
    # OPAD knocks the output row stride off the exact 64KB power of two:
    # with stride 2^16 every partition's descriptor hits the same HBM
    # channel/bank alias pattern, which shows up as uneven DMA-engine
    # throughput in the trace.
    out = nc.dram_tensor(
        "out", [O, K * BT + OPAD], _OUT_DT, kind="ExternalOutput"
    ).ap()

    assert sum(OUT_GROUPS) == K

    order = BAND_ORDER(K)

    # input segmentation, interleaved by first-use PROCESSING POSITION so
    # each transfer lands just before its consumers.  The first x/w
    # segments are small so position-0's compute can start early.
    pos_of = {k: p for p, k in enumerate(order)}
    ch_first = [K] * n_xch
    for k in range(K):
        for pc in pieces[k]:
            c = pc[0]
            ch_first[c] = min(ch_first[c], pos_of[k])

    def _segment(n_items, first_of, pbounds):
        """Split [0, n_items) into contiguous index ranges such that each
        range's min first-use position falls in successive position bands."""
        firsts = [first_of(i) for i in range(n_items)]
        segs = []
        taken = [False] * n_items
        for pb in pbounds:
            sel = [i for i in range(n_items) if not taken[i] and firsts[i] < pb]
            while sel:
                lo = hi = sel[0]
                while hi + 1 <= sel[-1] and (hi + 1) in sel:
                    hi += 1
                segs.append((min(firsts[lo : hi + 1]), lo, hi))
                for i in range(lo, hi + 1):
                    taken[i] = True
                sel = [i for i in sel if i > hi]
        return segs

    pbounds = (4, 12, 24, 40, K + 1)
    xsegs = _segment(n_xch, lambda c: ch_first[c], pbounds)
    wsegs = _segment(n_wcol, lambda j: wcol_first[j], pbounds)
    loads = sorted(
        [("x", i, fb) for i, (fb, _, _) in enumerate(xsegs)]
        + [("w", i, fb) for i, (fb, _, _) in enumerate(wsegs)],
        key=lambda t: (t[2], t[0] == "x"),
    )

    import contextlib

    with tile.TileContext(nc) as tc:
        with contextlib.ExitStack() as ctx:
            stage_pool = ctx.enter_context(
                tc.tile_pool(name="stage", bufs=len(OUT_GROUPS))
            )
            # 7 single-bank per-band PSUM tiles + 1 bank for warm-up/filler
            # dummies = all 8 banks.  Independent banks let each band's
            # copy retire independently (~3us of PE runway), instead of a
            # 2-band tile coupling both copy engines to one pair and
            # capping the drain at per-pair LATENCY (~1.0us) rather than
            # per-engine THROUGHPUT (~0.68us/band-pair combined).
            psum_pool = ctx.enter_context(
                tc.tile_pool(name="psum", bufs=7, space="PSUM")
            )
            warm_pool = ctx.enter_context(tc.tile_pool(name="warm", bufs=1))
            warmps_pool = ctx.enter_context(
                tc.tile_pool(name="warmps", bufs=1, space="PSUM")
            )
            if with_bias:
                bias_pool = ctx.enter_context(tc.tile_pool(name="bias", bufs=1))

            # --- PE warm-up + filler target: one dedicated PSUM bank takes
            # all dummy full-array matmuls (nothing reads it).  Warm-up
            # bridges the PE from first dispatch to the first real matmul;
            # the HAM clock gate lifts 1.2 -> 2.4 GHz after ~5.4us of
            # sustained high-utilization matmul activity, which the real
            # big-band stream then maintains.  Memset on GpSimd so the
            # first warm-up matmul issues as early as possible.
            wdum = warm_pool.tile([P, O + BT], _IN_DT)
            wps = warmps_pool.tile([O, BT], _F32)
            nc.gpsimd.memset(wdum[:, :], 0)

            def dummy_mm():
                nc.tensor.matmul(
                    wps[:, :],
                    wdum[:, :O],
                    wdum[:, O : O + BT],
                    start=True,
                    stop=True,
                    tile_position=(0, 0),
                )

            for _ in range(N_WARMUP):
                dummy_mm()

            if with_bias:
                bias_t = bias_pool.tile([O, K], _F32)
                nc.sync.dma_start(out=bias_t[:, :], in_=bt[:, :])

            # Input loads in consumption order, ALL on the Sync ring: one
            # ring guarantees FIFO transfer order, so position 0's small
            # x/w segments complete first.  (Issuing the first loads from
            # GpSimd's SWDGE ring started 1.2us earlier but re-skewed the
            # DMA engines and raced the Sync ring — net regression.)
            xtiles = [None] * len(xsegs)
            wtiles = [None] * len(wsegs)
            for li, (kind, i, _) in enumerate(loads):
                eng = nc.sync
                if kind == "x":
                    _, clo, chi = xsegs[i]
                    xp = ctx.enter_context(tc.tile_pool(name=f"xseg{i}", bufs=1))
                    xt = xp.tile([P, (chi - clo + 1) * BT], _IN_DT)
                    eng.dma_start(
                        out=xt[:, :], in_=xg[:, clo * BT : (chi + 1) * BT]
                    )
                    xtiles[i] = (xt, clo, chi)
                else:
                    _, wlo, whi = wsegs[i]
                    wp = ctx.enter_context(tc.tile_pool(name=f"wseg{i}", bufs=1))
                    wt = wp.tile([P, (whi - wlo + 1) * O], _IN_DT)
                    eng.dma_start(
                        out=wt[:, :], in_=wg[:, wlo * O : (whi + 1) * O]
                    )
                    wtiles[i] = (wt, wlo, whi)

            def xof(c):
                for (xt, clo, chi) in xtiles:
                    if clo <= c <= chi:
                        return xt, c - clo
                raise AssertionError(c)

            def wof(j):
                for (wt, wlo, whi) in wtiles:
                    if wlo <= j <= whi:
                        return wt, j - wlo
                raise AssertionError(j)

            p0i = 0
            for gsz in OUT_GROUPS:
                gbands = order[p0i : p0i + gsz]
                k_min = min(gbands)
                assert sorted(gbands) == list(range(k_min, k_min + gsz))
                stage = stage_pool.tile([O, gsz * BT], _OUT_DT, tag="stage")
                # One PSUM bank per band; each band's PSUM->SBUF cast-copy
                # alternates DVE/ACT by position so the two engines pipeline
                # independent bands concurrently.
                for j in range(gsz):
                    k = gbands[j]
                    pos = p0i + j
                    psum = psum_pool.tile([O, BT], _F32, tag="psum")
                    plist = pieces[k]
                    for pi, (c, a, p0, e, wcol, tgt) in enumerate(plist):
                        xt, lc = xof(c)
                        wt, wc = wof(wcol)
                        nc.tensor.matmul(
                            psum[:, :],
                            wt[a : a + tgt, wc * O : (wc + 1) * O],
                            xt[a : a + tgt, lc * BT : (lc + 1) * BT],
                            start=(pi == 0),
                            stop=(pi == len(plist) - 1),
                            tile_position=(a, 0),
                        )
                    dst = stage[:, (k - k_min) * BT : (k - k_min + 1) * BT]
                    if with_bias:
                        nc.vector.tensor_scalar_add(
                            out=dst, in0=psum[:, :], scalar1=bias_t[:, k : k + 1]
                        )
                    elif pos % 2 == 0:
                        nc.vector.tensor_copy(dst, psum[:, :])
                    else:
                        nc.scalar.copy(dst, psum[:, :])
                    if pos % 2 == 1 and pos < FILLER_POS:
                        dummy_mm()
                # GpSimd/SWDGE ring: keeps outputs off the Sync ring and
                # off the compute engines.
                nc.gpsimd.dma_start(
                    out=out[:, k_min * BT : (k_min + gsz) * BT],
                    in_=stage[:, :],
                )
                p0i += gsz
    _split_excess_waits(nc)
    return nc


_CACHE = {}
LAST_RESULTS = None


def kernel(x, idx, mel_w, pre_w, pre_b):
    global LAST_RESULTS
    x = np.ascontiguousarray(np.asarray(x, dtype=np.float32))
    pre_w = np.asarray(pre_w, dtype=np.float32)
    pre_b = np.asarray(pre_b, dtype=np.float32)
    mel_w = np.asarray(mel_w, dtype=np.float32)
    B, C, T, F = x.shape
    K = np.asarray(idx).shape[0]
    assert C == 2 and T % N_CORES == 0
    T_loc = T // N_CORES
    assert B * T_loc == BT and pre_w.shape[-1] == O and K == sum(OUT_GROUPS)

    starts, lengths = _band_structure(idx, mel_w)
    with_bias = bool(np.any(pre_b != 0.0))
    key = (B, C, T, F, K, with_bias, starts.tobytes(), lengths.tobytes())
    if key not in _CACHE:
        chunk_rows, pieces, n_wcol, wcol_first = _plan(starts, lengths, F)
        nc = _build_program(chunk_rows, pieces, n_wcol, wcol_first, K, with_bias)
        _CACHE[key] = (nc, chunk_rows, pieces, n_wcol)
    nc, chunk_rows, pieces, n_wcol = _CACHE[key]
    n_xch = len(chunk_rows)

    # ---- weights: fold mel into pre_w, interleave channels, pack columns ----
    wrows = np.zeros((n_wcol * P, O), dtype=np.float32)
    for k in range(K):
        n = int(lengths[k])
        w2 = mel_w[k, None, :n, None] * pre_w[k, :, :n, :]  # (C, n, O)
        stacked = w2.transpose(1, 0, 2).reshape(2 * n, O)   # rows (w, c)
        off = 0
        for (c, a, p0, e, wcol, tgt) in pieces[k]:
            nreal = e - p0
            wrows[wcol * P + p0 : wcol * P + e] = stacked[off : off + nreal]
            off += nreal
    wg = np.ascontiguousarray(
        wrows.reshape(n_wcol, P, O).transpose(1, 0, 2).reshape(P, n_wcol * O)
    ).astype(_IN_NP)

    btT = np.ascontiguousarray(pre_b.T)  # (O, K) fp32

    # ---- per-core x: channel-interleaved rows (2f+c) gathered per chunk ----
    # virtual row v = 2f + c; chunk cc takes rows [chunk_rows[cc], +128)
    row_idx = np.concatenate(
        [np.arange(r0, r0 + P) for r0 in chunk_rows]
    )  # (n_xch*P,)
    valid = row_idx < 2 * F
    row_idx_c = np.where(valid, row_idx, 0)
    in_maps = []
    for ci in range(N_CORES):
        sl = x[:, :, ci * T_loc : (ci + 1) * T_loc, :]  # (B, C, T_loc, F)
        xt3 = np.ascontiguousarray(sl.transpose(3, 1, 0, 2)).reshape(2 * F, BT)
        gath = xt3[row_idx_c]
        gath[~valid] = 0.0
        xgc = np.zeros((P, n_xch * BT + OPAD), dtype=_IN_NP)
        xgc[:, : n_xch * BT] = (
            gath.reshape(n_xch, P, BT).transpose(1, 0, 2).reshape(P, n_xch * BT)
        )
        m = {"xg": xgc, "wg": wg}
        if with_bias:
            m["bt"] = btT
        in_maps.append(m)

    trace = bool(os.environ.get("BANDSPLIT_TRACE"))
    if trace:
        trace = _install_trace_hook()
    res = bass_utils.run_bass_kernel_spmd(
        nc, in_maps, list(range(N_CORES)), trace=trace
    )
    LAST_RESULTS = res

    outs = np.stack(
        [
            np.asarray(res.results[ci]["out"], dtype=np.float32)[:, : K * BT]
            for ci in range(N_CORES)
        ],
        axis=0,
    )
    # (n_cores, O, K*BT) -> (n_cores, O, K, B, T_loc) -> (B, O, T, K)
    outs = outs.reshape(N_CORES, O, K, B, T_loc)
    full = outs.transpose(3, 1, 0, 4, 2).reshape(B, O, T, K)
    return np.ascontiguousarray(full)


# revision 37
# speedup vs baseline: 1.0907x; 1.0907x over previous
"""Trainium2 Bass kernel for nn_BandSplit (v3).

Computes, for each of K mel bands:
    out[b, o, t, k] = sum_{c,w} x[b, c, t, idx[k,w]] * mel_w[k,w] * pre_w[k,c,w,o] + pre_b[k,o]

Structure exploited:
  - Band indices idx[k, :n_k] are contiguous runs (triangular mel filters),
    so the gather is a strided slice.
  - mel_w folds into pre_w on the host: W2[k,c,w,o] = mel_w[k,w]*pre_w[k,c,w,o].
  - x rows are channel-interleaved (row = 2f + c); band k's contraction is
    the contiguous row run [2s_k, 2s_k+2n_k).

What got this from the 69-78us v2 baseline to ~48-53us (all findings
measured from perfetto traces of this kernel on TRN2):
  - v2's binding constraint was total DMA BYTES through the 16 shared DMA
    engines.  Output moves to bf16 (rel-err gate is 2e-2; bf16 rounding
    adds ~1e-3): halves the dominant output stream (16.8 -> 8.4 MB/core).
    Host upcasts.
  - The output row stride was exactly 64KB; with a power-of-two stride
    every partition's descriptor hits the same HBM channel alias and one
    DMA engine runs ~20% long, adding ~5-9us of straggler tail.  OPAD
    pads the stride off the alias.  (Out-DMA COUNT also matters: 9
    output DMAs spread descriptors evenly; 8 or 10 re-skew one engine.)
  - The HAM clock gate tracks PE ARRAY UTILIZATION (rows/128), not just
    activity: only sustained streams of high-row matmuls lift the PE
    from 1.2 to 2.4 GHz, and 32/64-row matmuls run 381-427ns regardless
    of clock while 96/128-row ones hit 216ns at speed.  So: bands run
    big-first (descending), every matmul is padded to >=96 rows from
    base 0 with zero weight rows (free inside the 32-row weight-packing
    granules), and dummy full-array matmuls interleave in the first 32
    positions to hold utilization while the pipeline fills.
  - x chunks are packed with bounded row duplication: a band crossing a
    128-row chunk boundary restarts a fresh chunk when the duplicated
    prefix is small, instead of splitting into two matmuls.
  - One PSUM bank per band (7 rotating + 1 for dummies); each band's
    PSUM->SBUF cast-copy alternates DVE/ACT so the two engines pipeline
    independent bands (~0.3us/band drain when saturated).
  - The entire per-core output is staged in SBUF (stage pool bufs =
    n_groups), so compute never blocks on the output stream.
  - All input DMAs issue on the one Sync ring in consumption order with
    fine segmentation (positions 4/12/24/40): ring FIFO guarantees the
    first bands' data lands first; cross-ring issuance races cost ~5us.
  - Warm-up memset on GpSimd; 3 warm-up matmuls bridge PE dispatch to
    the first real matmul.

Sharding: data-parallel over T across 8 cores (T=1024 -> 128/core); identical
SPMD program per core, weights replicated, host reassembles (B, O, T, K).
"""

import os
import sys
import types

import numpy as np

for _p in ("/opt/trn_rl_repo",):
    if _p not in sys.path:
        sys.path.insert(0, _p)

import ml_dtypes

import concourse.bass as bass
import concourse.mybir as mybir
import concourse.tile as tile
from concourse import bass_utils

N_CORES = 8
O = 128          # out channels (= stationary free dim = PSUM partitions)
P = 128          # SBUF partitions / chunk rows
BT = 512         # B * T_loc columns per core
N_WARMUP = int(os.environ.get("BANDSPLIT_WARMUP", "3"))
DUP_MAX = int(os.environ.get("BANDSPLIT_DUPMAX", "32"))
# bands per stage tile / output DMA: small first groups align with the
# first input segment (positions 0-3) so the first group never waits on a
# later segment's transfer.
OUT_GROUPS = [4, 4] + [8] * 7
OPAD = int(os.environ.get("BANDSPLIT_OPAD", "64"))  # output row-stride pad (elems)
# Bands run big-first (descending): wide bands' full-array matmuls hold the
# HAM clock gate's utilization threshold from the start, so the 2.4 GHz
# flip happens early and most of the stream runs at full clock.
BAND_ORDER = lambda K: list(range(K - 1, -1, -1))
# Dummy full-array matmuls interleave after pairs of bands processed at
# positions < FILLER_POS: when the drain (PSUM->SBUF copies) paces the
# pipeline, the PE would idle ~35% at 2.4 GHz and the HAM gate would
# re-throttle it (measured: the down-flip lands exactly where the fillers
# stop); the filler keeps utilization pinned through the whole stream.
FILLER_POS = int(os.environ.get("BANDSPLIT_FILLER_POS", "32"))

_F32 = mybir.dt.float32
_IN_DT = mybir.dt.bfloat16
_IN_NP = ml_dtypes.bfloat16

if os.environ.get("BANDSPLIT_OUT_DT", "bf16") == "f32":
    _OUT_DT = mybir.dt.float32
    _OUT_NP = np.float32
else:
    _OUT_DT = mybir.dt.bfloat16
    _OUT_NP = ml_dtypes.bfloat16


# ---------------------------------------------------------------------------
# Workaround: this container's walrus rejects instructions carrying more than
# a couple of sem waits ("Too many sync wait commands", CoreV3GenImpl
# setupSyncWait).  Post-pass: move excess waits onto single-wait NoOps
# inserted just before the instruction on the same engine/sequencer.
# ---------------------------------------------------------------------------
_MAX_WAITS = 1

if os.environ.get("BANDSPLIT_LDWOPT"):
    # Experiment: let walrus overlap LDWEIGHTS with matmuls (the default
    # pipeline pins --enable-ldw-opt=false).
    _orig_run_command = bass_utils.run_command

    def _patched_run_command(cmd, **kw):
        if isinstance(cmd, list):
            cmd = [
                "--enable-ldw-opt=true" if c == "--enable-ldw-opt=false" else c
                for c in cmd
            ]
        return _orig_run_command(cmd, **kw)

    bass_utils.run_command = _patched_run_command


def _split_excess_waits(nc, max_waits=_MAX_WAITS):
    ctr = 0
    for f in nc.m.functions:
        for bb in f.blocks:
            il = bb.instructions
            i = 0
            while i < len(il):
                inst = il[i]
                si = inst.sync_info
                if si is not None and si.on_wait and len(si.on_wait) > max_waits:
                    waits = list(si.on_wait)
                    keep = waits[-max_waits:]
                    extra = waits[:-max_waits]
                    nops = []
                    for w in extra:
                        ctr += 1
                        nop = mybir.InstNoOp(
                            name=f"{inst.name}-wsplit{ctr}",
                            engine=inst.engine,
                            sync_info=mybir.SyncInfo(on_wait=[w], on_update=[]),
                            bass_nofuse=True,
                        )
                        nc.register_instruction(nop, overwrite=True)
                        nops.append(nop)
                    inst.sync_info = mybir.SyncInfo(
                        on_wait=keep, on_update=list(si.on_update or [])
                    )
                    il[i:i] = nops
                    i += len(nops)
                i += 1
    return ctr


# ---------------------------------------------------------------------------
# Optional NTFF profiling (test.py sets BANDSPLIT_TRACE=1).  The agent image's
# antenv lacks axon_hooks, so tracing degrades silently unless we install the
# ctypes-based hook ourselves.
# ---------------------------------------------------------------------------
def _install_trace_hook():
    try:
        import antenv  # noqa: F401
        from trn_agent_boot.trn_boot import _ntff_profile_via_ctypes

        if "antenv.axon_hooks" in sys.modules:
            return True
        hook = _ntff_profile_via_ctypes("/opt/axon/libaxon_pjrt.so")
        mod = types.ModuleType("antenv.axon_hooks")
        mod._hook = hook
        mod.get_axon_ntff_profile_hook = lambda: mod._hook
        mod.set_axon_ntff_profile_hook = lambda h: setattr(mod, "_hook", h)
        sys.modules["antenv.axon_hooks"] = mod
        import antenv as _ae

        _ae.axon_hooks = mod
        return True
    except Exception:
        return False


# ---------------------------------------------------------------------------
# Band structure extraction (host side, from the actual inputs)
# ---------------------------------------------------------------------------
def _band_structure(idx, mel_w):
    idx = np.asarray(idx)
    mel_w = np.asarray(mel_w)
    K = idx.shape[0]
    starts = np.empty(K, dtype=np.int64)
    lengths = np.empty(K, dtype=np.int64)
    for k in range(K):
        nz = np.nonzero(mel_w[k])[0]
        assert nz.size > 0, f"band {k} empty"
        n = int(nz.max()) + 1
        run = idx[k, :n]
        assert np.all(np.diff(run) == 1), f"band {k} indices not contiguous"
        starts[k] = int(run[0])
        lengths[k] = n
    return starts, lengths


def _align_base(p0, e):
    """Largest legal 32-aligned base <= p0 for a piece ending at e.

    tile_position rule: rows<=32 -> base in {0,32,64,96}; rows<=64 -> {0,64};
    rows>64 -> base 0.
    """
    for a in (96, 64, 32, 0):
        if a > p0:
            continue
        rows = e - a
        if rows <= 32 or (rows <= 64 and a in (0, 64)) or a == 0:
            return a
    raise AssertionError((p0, e))


# HW note: nonzero tile_position row bases are only safe for single-matmul
# bands (start=stop=True).  Mixing bases inside a PSUM accumulation group
# (split bands) aborts the NEFF at runtime on this stack, so split bands'
# pieces all use base 0 (with zero weight rows below p0).


def _plan(starts, lengths, F):
    """Pack band row-runs into 128-row chunks with bounded duplication.

    Chunks are arbitrary 128-row windows of the channel-interleaved row
    space, created in increasing start order.  A band crossing the current
    window restarts a fresh window at its own first row when the duplicated
    prefix is <= DUP_MAX rows (one matmul instead of two); otherwise it
    splits along the natural continuation grid.

    Returns:
      chunk_rows  -> list of chunk start rows (virtual row space, 2F wide)
      pieces[k]   -> list of (chunk, base, p0, e, wcol); rows [base,p0) are
                     zero weight extension, [p0,e) real
      n_wcol      -> number of packed 128-row weight columns
      wcol_first  -> first band using each weight column (ascending)
    """
    K = len(starts)
    chunks = []
    raw = []
    for k in range(K):
        a = 2 * int(starts[k])
        b = a + 2 * int(lengths[k])
        pl = []
        ci = None
        for i in range(len(chunks) - 1, -1, -1):
            if chunks[i] <= a:
                ci = i
                break
        if ci is None or a >= chunks[ci] + P:
            chunks.append(a)
            ci = len(chunks) - 1
        if b <= chunks[ci] + P:
            pl.append((ci, a - chunks[ci], b - a))
        else:
            dup = chunks[ci] + P - a
            if b - a <= P and dup <= DUP_MAX:
                chunks.append(a)
                ci = len(chunks) - 1
                pl.append((ci, 0, b - a))
            else:
                r = a
                while r < b:
                    if r >= chunks[ci] + P:
                        if ci + 1 < len(chunks) and chunks[ci + 1] <= r:
                            ci += 1
                        else:
                            chunks.append(chunks[ci] + P)
                            ci = len(chunks) - 1
                    e = min(b, chunks[ci] + P)
                    pl.append((ci, r - chunks[ci], e - r))
                    r = e
        raw.append(pl)

    # Weight column packing: first-fit on 32-row granules, in PROCESSING
    # order (descending bands), so early columns hold early-consumed
    # weights and column ranges stream in consumption order.
    #
    # Matmul rows are PADDED up to the 32-granule class (tgt): the HAM
    # clock gate tracks PE array utilization (rows streamed / 128), and
    # only sustained high-utilization matmul activity lifts the PE from
    # 1.2 to 2.4 GHz.  The pad rows are zeros already inside the piece's
    # reserved granules, so this costs no extra weight bytes.
    pieces = [[] for _ in range(K)]
    col_fill = []
    wcol_first = []
    for pos, k in enumerate(BAND_ORDER(K)):
        single = len(raw[k]) == 1
        for (c, p0, rows) in raw[k]:
            e = p0 + rows
            # Every matmul streams >=96 rows from base 0: measured on HW,
            # 96/128-row matmuls run 261ns at 2.4 GHz while 32/64-row ones
            # run 381ns AND fail to hold the HAM utilization gate.  The pad
            # rows are zero weight rows; base 0 keeps tile_position legal
            # for any row count.
            a0 = 0
            tgt = 96 if e <= 96 else P
            s_lo, s_hi = 0, tgt // 32
            wcol = None
            for j in range(len(col_fill)):
                if col_fill[j] <= s_lo:
                    wcol = j
                    col_fill[j] = s_hi
                    break
            if wcol is None:
                col_fill.append(s_hi)
                wcol_first.append(pos)
                wcol = len(col_fill) - 1
            pieces[k].append((c, a0, p0, e, wcol, tgt))
    return chunks, pieces, len(col_fill), wcol_first


# ---------------------------------------------------------------------------
# Device program
# ---------------------------------------------------------------------------
def _build_program(chunk_rows, pieces, n_wcol, wcol_first, K, with_bias):
    nc = bass.Bass("TRN2", target_bir_lowering=False, debug=False)
    n_xch = len(chunk_rows)
    xg = nc.dram_tensor("xg", [P, n_xch * BT], _IN_DT, kind="ExternalInput").ap()
    wg = nc.dram_tensor("wg", [P, n_wcol * O], _IN_DT, kind="ExternalInput").ap()
    if with_bias:
        bt = nc.dram_tensor("bt", [O, K], _F32, kind="ExternalInput").ap()
    # OPAD knocks the output row stride off the exact 64KB power of two:
    # with stride 2^16 every partition's descriptor hits the same HBM
    # channel/bank alias pattern, which shows up as uneven DMA-engine
    # throughput in the trace.
    out = nc.dram_tensor(
        "out", [O, K * BT + OPAD], _OUT_DT, kind="ExternalOutput"
    ).ap()

    assert sum(OUT_GROUPS) == K

    order = BAND_ORDER(K)

    # input segmentation, interleaved by first-use PROCESSING POSITION so
    # each transfer lands just before its consumers.  The first x/w
    # segments are small so position-0's compute can start early.
    pos_of = {k: p for p, k in enumerate(order)}
    ch_first = [K] * n_xch
    for k in range(K):
        for pc in pieces[k]:
            c = pc[0]
            ch_first[c] = min(ch_first[c], pos_of[k])

    def _segment(n_items, first_of, pbounds):
        """Split [0, n_items) into contiguous index ranges such that each
        range's min first-use position falls in successive position bands."""
        firsts = [first_of(i) for i in range(n_items)]
        segs = []
        taken = [False] * n_items
        for pb in pbounds:
            sel = [i for i in range(n_items) if not taken[i] and firsts[i] < pb]
            while sel:
                lo = hi = sel[0]
                while hi + 1 <= sel[-1] and (hi + 1) in sel:
                    hi += 1
                segs.append((min(firsts[lo : hi + 1]), lo, hi))
                for i in range(lo, hi + 1):
                    taken[i] = True
                sel = [i for i in sel if i > hi]
        return segs

    pbounds = (4, 12, 24, 40, K + 1)
    xsegs = _segment(n_xch, lambda c: ch_first[c], pbounds)
    wsegs = _segment(n_wcol, lambda j: wcol_first[j], pbounds)
    loads = sorted(
        [("x", i, fb) for i, (fb, _, _) in enumerate(xsegs)]
        + [("w", i, fb) for i, (fb, _, _) in enumerate(wsegs)],
        key=lambda t: (t[2], t[0] == "x"),
    )

    import contextlib

    with tile.TileContext(nc) as tc:
        with contextlib.ExitStack() as ctx:
            stage_pool = ctx.enter_context(
                tc.tile_pool(name="stage", bufs=len(OUT_GROUPS))
            )
            # 7 single-bank per-band PSUM tiles + 1 bank for warm-up/filler
            # dummies = all 8 banks.  Independent banks let each band's
            # copy retire independently (~3us of PE runway), instead of a
            # 2-band tile coupling both copy engines to one pair and
            # capping the drain at per-pair LATENCY (~1.0us) rather than
            # per-engine THROUGHPUT (~0.68us/band-pair combined).
            psum_pool = ctx.enter_context(
                tc.tile_pool(name="psum", bufs=7, space="PSUM")
            )
            warm_pool = ctx.enter_context(tc.tile_pool(name="warm", bufs=1))
            warmps_pool = ctx.enter_context(
                tc.tile_pool(name="warmps", bufs=1, space="PSUM")
            )
            if with_bias:
                bias_pool = ctx.enter_context(tc.tile_pool(name="bias", bufs=1))

            # --- PE warm-up + filler target: one dedicated PSUM bank takes
            # all dummy full-array matmuls (nothing reads it).  Warm-up
            # bridges the PE from first dispatch to the first real matmul;
            # the HAM clock gate lifts 1.2 -> 2.4 GHz after ~5.4us of
            # sustained high-utilization matmul activity, which the real
            # big-band stream then maintains.  Memset on GpSimd so the
            # first warm-up matmul issues as early as possible.
            wdum = warm_pool.tile([P, O + BT], _IN_DT)
            wps = warmps_pool.tile([O, BT], _F32)
            nc.gpsimd.memset(wdum[:, :], 0)

            def dummy_mm():
                nc.tensor.matmul(
                    wps[:, :],
                    wdum[:, :O],
                    wdum[:, O : O + BT],
                    start=True,
                    stop=True,
                    tile_position=(0, 0),
                )

            for _ in range(N_WARMUP):
                dummy_mm()

            if with_bias:
                bias_t = bias_pool.tile([O, K], _F32)
                nc.sync.dma_start(out=bias_t[:, :], in_=bt[:, :])

            # Input loads in consumption order, ALL on the Sync ring: one
            # ring guarantees FIFO transfer order, so position 0's small
            # x/w segments complete first.  (Issuing the first loads from
            # GpSimd's SWDGE ring started 1.2us earlier but re-skewed the
            # DMA engines and raced the Sync ring — net regression.)
            xtiles = [None] * len(xsegs)
            wtiles = [None] * len(wsegs)
            for li, (kind, i, _) in enumerate(loads):
                eng = nc.sync
                if kind == "x":
                    _, clo, chi = xsegs[i]
                    xp = ctx.enter_context(tc.tile_pool(name=f"xseg{i}", bufs=1))
                    xt = xp.tile([P, (chi - clo + 1) * BT], _IN_DT)
                    eng.dma_start(
                        out=xt[:, :], in_=xg[:, clo * BT : (chi + 1) * BT]
                    )
                    xtiles[i] = (xt, clo, chi)
                else:
                    _, wlo, whi = wsegs[i]
                    wp = ctx.enter_context(tc.tile_pool(name=f"wseg{i}", bufs=1))
                    wt = wp.tile([P, (whi - wlo + 1) * O], _IN_DT)
                    eng.dma_start(
                        out=wt[:, :], in_=wg[:, wlo * O : (whi + 1) * O]
                    )
                    wtiles[i] = (wt, wlo, whi)

            def xof(c):
                for (xt, clo, chi) in xtiles:
                    if clo <= c <= chi:
                        return xt, c - clo
                raise AssertionError(c)

            def wof(j):
                for (wt, wlo, whi) in wtiles:
                    if wlo <= j <= whi:
                        return wt, j - wlo
                raise AssertionError(j)

            p0i = 0
            for gsz in OUT_GROUPS:
                gbands = order[p0i : p0i + gsz]
                k_min = min(gbands)
                assert sorted(gbands) == list(range(k_min, k_min + gsz))
                stage = stage_pool.tile([O, gsz * BT], _OUT_DT, tag="stage")
                # One PSUM bank per band; each band's PSUM->SBUF cast-copy
                # alternates DVE/ACT by position so the two engines pipeline
                # independent bands concurrently.
                for j in range(gsz):
                    k = gbands[j]
                    pos = p0i + j
                    psum = psum_pool.tile([O, BT], _F32, tag="psum")
                    plist = pieces[k]
                    for pi, (c, a, p0, e, wcol, tgt) in enumerate(plist):
                        xt, lc = xof(c)
                        wt, wc = wof(wcol)
                        nc.tensor.matmul(
                            psum[:, :],
                            wt[a : a + tgt, wc * O : (wc + 1) * O],
                            xt[a : a + tgt, lc * BT : (lc + 1) * BT],
                            start=(pi == 0),
                            stop=(pi == len(plist) - 1),
                            tile_position=(a, 0),
                        )
                    dst = stage[:, (k - k_min) * BT : (k - k_min + 1) * BT]
                    if with_bias:
                        nc.vector.tensor_scalar_add(
                            out=dst, in0=psum[:, :], scalar1=bias_t[:, k : k + 1]
                        )
                    elif pos % 2 == 0:
                        nc.vector.tensor_copy(dst, psum[:, :])
                    else:
                        nc.scalar.copy(dst, psum[:, :])
                    if pos % 2 == 1 and pos < FILLER_POS:
                        dummy_mm()
                # GpSimd/SWDGE ring: keeps outputs off the Sync ring and
                # off the compute engines.
                nc.gpsimd.dma_start(
                    out=out[:, k_min * BT : (k_min + gsz) * BT],
                    in_=stage[:, :],
                )
                p0i += gsz
    _split_excess_waits(nc)
    return nc


_CACHE = {}
LAST_RESULTS = None


def kernel(x, idx, mel_w, pre_w, pre_b):
    global LAST_RESULTS
    x = np.ascontiguousarray(np.asarray(x, dtype=np.float32))
    pre_w = np.asarray(pre_w, dtype=np.float32)
    pre_b = np.asarray(pre_b, dtype=np.float32)
    mel_w = np.asarray(mel_w, dtype=np.float32)
    B, C, T, F = x.shape
    K = np.asarray(idx).shape[0]
    assert C == 2 and T % N_CORES == 0
    T_loc = T // N_CORES
    assert B * T_loc == BT and pre_w.shape[-1] == O and K == sum(OUT_GROUPS)

    starts, lengths = _band_structure(idx, mel_w)
    with_bias = bool(np.any(pre_b != 0.0))
    key = (B, C, T, F, K, with_bias, starts.tobytes(), lengths.tobytes())
    if key not in _CACHE:
        chunk_rows, pieces, n_wcol, wcol_first = _plan(starts, lengths, F)
        nc = _build_program(chunk_rows, pieces, n_wcol, wcol_first, K, with_bias)
        _CACHE[key] = (nc, chunk_rows, pieces, n_wcol)
    nc, chunk_rows, pieces, n_wcol = _CACHE[key]
    n_xch = len(chunk_rows)

    # ---- weights: fold mel into pre_w, interleave channels, pack columns ----
    wrows = np.zeros((n_wcol * P, O), dtype=np.float32)
    for k in range(K):
        n = int(lengths[k])
        w2 = mel_w[k, None, :n, None] * pre_w[k, :, :n, :]  # (C, n, O)
        stacked = w2.transpose(1, 0, 2).reshape(2 * n, O)   # rows (w, c)
        off = 0
        for (c, a, p0, e, wcol, tgt) in pieces[k]:
            nreal = e - p0
            wrows[wcol * P + p0 : wcol * P + e] = stacked[off : off + nreal]
            off += nreal
    wg = np.ascontiguousarray(
        wrows.reshape(n_wcol, P, O).transpose(1, 0, 2).reshape(P, n_wcol * O)
    ).astype(_IN_NP)

    btT = np.ascontiguousarray(pre_b.T)  # (O, K) fp32

    # ---- per-core x: channel-interleaved rows (2f+c) gathered per chunk ----
    # virtual row v = 2f + c; chunk cc takes rows [chunk_rows[cc], +128)
    row_idx = np.concatenate(
        [np.arange(r0, r0 + P) for r0 in chunk_rows]
    )  # (n_xch*P,)
    valid = row_idx < 2 * F
    row_idx_c = np.where(valid, row_idx, 0)
    in_maps = []
    for ci in range(N_CORES):
        sl = x[:, :, ci * T_loc : (ci + 1) * T_loc, :]  # (B, C, T_loc, F)
        xt3 = np.ascontiguousarray(sl.transpose(3, 1, 0, 2)).reshape(2 * F, BT)
        gath = xt3[row_idx_c]
        gath[~valid] = 0.0
        xgc = np.ascontiguousarray(
            gath.reshape(n_xch, P, BT).transpose(1, 0, 2).reshape(P, n_xch * BT)
        ).astype(_IN_NP)
        m = {"xg": xgc, "wg": wg}
        if with_bias:
            m["bt"] = btT
        in_maps.append(m)

    trace = bool(os.environ.get("BANDSPLIT_TRACE"))
    if trace:
        trace = _install_trace_hook()
    res = bass_utils.run_bass_kernel_spmd(
        nc, in_maps, list(range(N_CORES)), trace=trace
    )
    LAST_RESULTS = res

    outs = np.stack(
        [
            np.asarray(res.results[ci]["out"], dtype=np.float32)[:, : K * BT]
            for ci in range(N_CORES)
        ],
        axis=0,
    )
    # (n_cores, O, K*BT) -> (n_cores, O, K, B, T_loc) -> (B, O, T, K)
    outs = outs.reshape(N_CORES, O, K, B, T_loc)
    full = outs.transpose(3, 1, 0, 4, 2).reshape(B, O, T, K)
    return np.ascontiguousarray(full)
